# revision 6
# baseline (speedup 1.0000x reference)
"""CAM (channel attention) module kernel for Trainium2, 8-core data-parallel.

Reference computation (per sample b):
    q = conv2d(x, Wq, stride2, 2x2) -> [C, 4096]
    k = conv2d(x, Wk, stride2, 2x2) -> [C, 4096]
    v = conv2d(x, Wv, 1x1)          -> [C, 16384]
    E = q @ k^T                      [C, C]
    att = softmax(rowmax(E) - E)   (== softmin over rows)
    out = att @ v -> [C, H, W]

Kernel strategy (one sample per NeuronCore, B=8 over 8 cores):
  - conv q/k as 4 accumulating matmuls (one per 2x2 tap) with strided
    im2col access patterns straight out of the resident x tile in SBUF.
    Producing q in [c, n] layout, then PE-transposing to [n, c] chunks
    for the energy contraction.
  - E accumulated over 32 [128]x[128,128] chunk matmuls in one PSUM bank.
  - softmax via one DVE row-min + one fused ScalarE exp (bias=rowmin,
    scale=-1) with accumulated row-sum, then scale by reciprocal.
  - out = att @ (Wv x + bv) == (att Wv) @ x + (att bv): computes the tiny
    [128,128] matrix M^T = Wv^T att^T on PE, then a single [c2 -> c,n]
    matmul against the resident x tile. This halves the output-path
    matmul work vs materializing v.
All matmuls in fp32 (the softmax here is extremely peaked -- energy
entries span hundreds -- so bf16/tf32 energy errors get exponentially
amplified; measured 0.28 / 0.03 rel-err vs 2.6e-5 for fp32).
"""

import numpy as np

B, C, H, W = 8, 128, 128, 128
HW = H * W           # 16384
N_CORES = 8
NB = 8               # number of H-bands (16 input rows each) == x/out chunks
BAND = HW // NB      # 2048 columns per band
QN = (H // 2) * (W // 2)  # 4096 conv output positions
QCHUNK = QN // NB    # 512 conv outputs per band

_CACHE = {}


def _build_program(with_qk_bias: bool, with_v_bias: bool):
    import concourse.tile as tile
    from concourse import bacc, mybir
    from concourse.masks import make_identity

    f32 = mybir.dt.float32
    nc = bacc.Bacc(
        "TRN2", target_bir_lowering=False, debug=False, num_devices=N_CORES)

    x_d = nc.declare_dram_parameter("x", [C, HW], f32, isOutput=False)
    wqT_d = nc.declare_dram_parameter("wqT", [C, 4 * C], f32, isOutput=False)
    wkT_d = nc.declare_dram_parameter("wkT", [C, 4 * C], f32, isOutput=False)
    wv_d = nc.declare_dram_parameter("wv", [C, C], f32, isOutput=False)
    if with_qk_bias:
        bq_d = nc.declare_dram_parameter("bq", [C, 1], f32, isOutput=False)
        bk_d = nc.declare_dram_parameter("bk", [C, 1], f32, isOutput=False)
    if with_v_bias:
        bv_d = nc.declare_dram_parameter("bv", [C, 1], f32, isOutput=False)
    out_d = nc.declare_dram_parameter("out", [C, HW], f32, isOutput=True)

    with tile.TileContext(nc) as tc:
        with (
            tc.tile_pool(name="const", bufs=1) as const,
            tc.tile_pool(name="xp", bufs=1) as xp,
            tc.tile_pool(name="qkT", bufs=1) as qkT,
            tc.tile_pool(name="stage", bufs=3) as stage,
            tc.tile_pool(name="small", bufs=2) as small,
            tc.tile_pool(name="pacc", bufs=3, space="PSUM") as pacc,
            tc.tile_pool(name="ptp", bufs=2, space="PSUM") as ptp,
            tc.tile_pool(name="psm", bufs=1, space="PSUM") as psm,
        ):
            ident = const.tile([128, 128], f32, tag="ident")
            make_identity(nc, ident)

            wqT_sb = const.tile([C, 4 * C], f32, tag="wqT")
            nc.sync.dma_start(out=wqT_sb, in_=wqT_d[:, :])
            wkT_sb = const.tile([C, 4 * C], f32, tag="wkT")
            nc.sync.dma_start(out=wkT_sb, in_=wkT_d[:, :])
            wv_sb = const.tile([C, C], f32, tag="wv")
            nc.sync.dma_start(out=wv_sb, in_=wv_d[:, :])
            if with_qk_bias:
                bq_sb = const.tile([C, 1], f32, tag="bq")
                nc.sync.dma_start(out=bq_sb, in_=bq_d[:, :])
                bk_sb = const.tile([C, 1], f32, tag="bk")
                nc.sync.dma_start(out=bk_sb, in_=bk_d[:, :])
            if with_v_bias:
                bv_sb = const.tile([C, 1], f32, tag="bv")
                nc.sync.dma_start(out=bv_sb, in_=bv_d[:, :])

            x_sb = []
            for j in range(NB):
                t = xp.tile([C, BAND], f32, tag=f"x{j}")
                nc.sync.dma_start(out=t, in_=x_d[:, j * BAND:(j + 1) * BAND])
                x_sb.append(t)

            def conv_qkT(wT_sb, bias_sb, name):
                """Returns list of NB SBUF tiles [128, 512] holding the
                transposed conv output: tile j, columns [t*128, t*128+128)
                hold (q^T)[n, c] for n in [512j+128t, 512j+128t+128)."""
                T_out = [qkT.tile([128, QCHUNK], f32, tag=f"{name}T{j}",
                                   name=f"{name}T{j}")
                         for j in range(NB)]
                pend = []  # (j, q_chunk_sbuf) awaiting PE transpose

                def emit_transpose(j, qc):
                    tp = ptp.tile([128, QCHUNK], f32, tag="tp")
                    for t in range(4):
                        nc.tensor.transpose(
                            tp[:, t * 128:(t + 1) * 128],
                            qc[:, t * 128:(t + 1) * 128],
                            ident,
                        )
                    nc.vector.tensor_copy(T_out[j], tp)

                for j in range(NB):
                    acc = pacc.tile([128, QCHUNK], f32, tag="acc")
                    xv = x_sb[j][:].rearrange(
                        "p (i a w b) -> p i a w b", i=8, a=2, w=64, b=2)
                    for ab in range(4):
                        a, bb = ab // 2, ab % 2
                        nc.tensor.matmul(
                            acc,
                            lhsT=wT_sb[:, ab * C:(ab + 1) * C],
                            rhs=xv[:, :, a, :, bb],
                            start=(ab == 0),
                            stop=(ab == 3),
                        )
                    qc = stage.tile([128, QCHUNK], f32, tag="qchunk")
                    if bias_sb is not None:
                        nc.scalar.activation(
                            out=qc, in_=acc,
                            func=mybir.ActivationFunctionType.Identity,
                            bias=bias_sb[:, 0:1], scale=1.0)
                    else:
                        nc.vector.tensor_copy(qc, acc)
                    pend.append((j, qc))
                    # emit transposes one band behind so PE never waits on
                    # the PSUM->SBUF copy of the band it just produced
                    if len(pend) > 1:
                        emit_transpose(*pend.pop(0))
                emit_transpose(*pend.pop(0))
                return T_out

            qT = conv_qkT(wqT_sb, bq_sb if with_qk_bias else None, "q")
            kT = conv_qkT(wkT_sb, bk_sb if with_qk_bias else None, "k")

            # energy E[c, d] accumulated over all 32 n-chunks
            E = psm.tile([128, 128], f32, tag="E")
            nchunks = NB * 4
            idx = 0
            for j in range(NB):
                for t in range(4):
                    nc.tensor.matmul(
                        E,
                        lhsT=qT[j][:, t * 128:(t + 1) * 128],
                        rhs=kT[j][:, t * 128:(t + 1) * 128],
                        start=(idx == 0),
                        stop=(idx == nchunks - 1),
                    )
                    idx += 1

            # softmax(rowmax - E) over rows == softmin: exp(rowmin - E) / Z
            mmin = small.tile([128, 1], f32, tag="mmin")
            nc.vector.tensor_reduce(
                out=mmin, in_=E, axis=mybir.AxisListType.X,
                op=mybir.AluOpType.min)
            w_sb = small.tile([128, 128], f32, tag="w")
            zsum = small.tile([128, 1], f32, tag="z")
            nc.scalar.activation(
                out=w_sb, in_=E, func=mybir.ActivationFunctionType.Exp,
                bias=mmin[:, 0:1], scale=-1.0, accum_out=zsum[:, 0:1])
            rz = small.tile([128, 1], f32, tag="rz")
            nc.vector.reciprocal(rz, zsum)
            att = small.tile([128, 128], f32, tag="att")
            nc.vector.tensor_scalar_mul(att, w_sb, rz[:, 0:1])

            # attT = att^T  (needed as the moving operand for M^T)
            attT_p = psm.tile([128, 128], f32, tag="s2")
            nc.tensor.transpose(attT_p, att, ident)
            attT = small.tile([128, 128], f32, tag="attT")
            nc.vector.tensor_copy(attT, attT_p)

            # M^T[c2, c] = sum_d Wv[d, c2] attT[d, c]
            MT_p = psm.tile([128, 128], f32, tag="s2")
            nc.tensor.matmul(MT_p, lhsT=wv_sb, rhs=attT, start=True, stop=True)
            MT = small.tile([128, 128], f32, tag="MT")
            nc.vector.tensor_copy(MT, MT_p)

            if with_v_bias:
                # abv[c] = sum_d att[c, d] bv[d]
                abv_p = psm.tile([128, 1], f32, tag="s2")
                nc.tensor.matmul(abv_p, lhsT=attT, rhs=bv_sb[:, 0:1],
                                 start=True, stop=True)
                abv = small.tile([128, 1], f32, tag="abv")
                nc.vector.tensor_copy(abv, abv_p)

            # out[c, n] = sum_c2 M[c, c2] x[c2, n] (+ abv[c])
            for j in range(NB):
                for s in range(BAND // 512):
                    o_p = pacc.tile([128, 512], f32, tag="acc")
                    nc.tensor.matmul(
                        o_p, lhsT=MT,
                        rhs=x_sb[j][:, s * 512:(s + 1) * 512],
                        start=True, stop=True)
                    o_sb = stage.tile([128, 512], f32, tag="ostage")
                    if with_v_bias:
                        nc.scalar.activation(
                            out=o_sb, in_=o_p,
                            func=mybir.ActivationFunctionType.Identity,
                            bias=abv[:, 0:1], scale=1.0)
                    else:
                        nc.vector.tensor_copy(o_sb, o_p)
                    off = j * BAND + s * 512
                    nc.sync.dma_start(out=out_d[:, off:off + 512], in_=o_sb)

    nc.compile()
    return nc


def kernel(x, Wq, bq, Wk, bk, Wv, bv):
    from concourse.bass_utils import run_bass_kernel_spmd

    x = np.ascontiguousarray(np.asarray(x, dtype=np.float32))
    Wq = np.asarray(Wq, dtype=np.float32)
    Wk = np.asarray(Wk, dtype=np.float32)
    Wv = np.asarray(Wv, dtype=np.float32)
    bq = np.asarray(bq, dtype=np.float32)
    bk = np.asarray(bk, dtype=np.float32)
    bv = np.asarray(bv, dtype=np.float32)

    with_qk_bias = bool(np.any(bq) or np.any(bk))
    with_v_bias = bool(np.any(bv))

    key = (with_qk_bias, with_v_bias)
    if key not in _CACHE:
        _CACHE[key] = _build_program(with_qk_bias, with_v_bias)
    nc = _CACHE[key]

    # weight layout prep: wT[cin, ab*128 + c] = W[c, cin, a, b]
    wqT = np.ascontiguousarray(
        Wq.transpose(1, 2, 3, 0).reshape(C, 4 * C))
    wkT = np.ascontiguousarray(
        Wk.transpose(1, 2, 3, 0).reshape(C, 4 * C))
    wv = np.ascontiguousarray(Wv.reshape(C, C))

    in_maps = []
    for b in range(B):
        m = {
            "x": np.ascontiguousarray(x[b].reshape(C, HW)),
            "wqT": wqT,
            "wkT": wkT,
            "wv": wv,
        }
        if with_qk_bias:
            m["bq"] = np.ascontiguousarray(bq.reshape(C, 1))
            m["bk"] = np.ascontiguousarray(bk.reshape(C, 1))
        if with_v_bias:
            m["bv"] = np.ascontiguousarray(bv.reshape(C, 1))
        in_maps.append(m)

    res = run_bass_kernel_spmd(nc, in_maps, list(range(N_CORES)))
    out = np.stack([res.results[i]["out"] for i in range(N_CORES)])
    return out.reshape(B, C, H, W).astype(np.float32)
